# revision 16
# baseline (speedup 1.0000x reference)
"""Dense 2-layer 2-head GAT for Trainium2 (Bass/Tile), data-parallel over batch.

v2 — rank-1 score factorization. Per head the score matrix is
s[i,j] = lrelu(hl_i + hr_j), so

  u = exp(lrelu(s)) = max(exp(s), exp(0.2 s))
    = max(outer(e^{hr}, e^{hl}), outer(e^{0.2 hr}, e^{0.2 hl}))

i.e. an elementwise max of two rank-1 outer products.  hl/hr come from one
tiny [4, n] matmul per layer with host-folded wa = W @ a, so score-tile
generation needs only TWO elementwise passes per tile:

  pass A:  t1 = e^{s}  — either ACT Exp(hl_bcast + hr_j bias)  (per-partition
           bias), or DVE tensor_scalar A_bcast * B_j (4x bf16 mode);
           the per-jc assignment is static to balance ACT vs DVE.
  pass B:  u = DVE stt (C_bcast * D_j) max t1   (2x bf16 mode)

Row-vector broadcasts (hl, A=e^hl, C=e^{0.2 hl}, 1/Z) are materialized by
DMA (idle engine) instead of PE broadcast matmuls.  The diagonal mask is
applied at the source (u diag = 0) by gpsimd affine_select, so no
numerator/denominator correction pass is needed; softmax's +identity is the
v2 = v + hT add.  elu is carried as elu+1 (saves a pass); layer-2 params are
host-adjusted (b2' = b2 - colsum(W2), score-bias consts) and the final store
subtracts 1 during the transpose copy-out.

Z (softmax denominator) is a [128,2]-stationary bf16 ones-matmul on the PE;
u and the h stationary operand are bf16 (scores themselves stay fp32).
"""

import os
from contextlib import ExitStack

import numpy as np

import concourse.bass as bass
import concourse.mybir as mybir
import concourse.tile as tile
from concourse.alu_op_type import AluOpType
from concourse.masks import make_identity

F32 = mybir.dt.float32
F32R = mybir.dt.float32r
BF16 = mybir.dt.bfloat16
AF = mybir.ActivationFunctionType

N = 2048
F = 256
D = 128
P = 128
ALPHA = 0.2
N_CORES = 8

NJ = N // P          # 16 j-chunks
IB = 512             # PSUM bank free width (fp32)
HW = 1024            # i-half width
NH = N // HW         # 2 halves
KH = HW // IB        # 2 k-blocks per half

# which jc tiles compute t1 on ACT (Exp w/ per-partition bias) vs DVE
# (A_bcast * B_j tensor_scalar).  Tuned to balance ACT vs DVE load.
ACT_T1 = tuple(jc for jc in range(NJ) if jc % 2 == 1)


def build_nc(n=N):
    from concourse import bacc
    nc = bacc.Bacc("TRN2", target_bir_lowering=False, debug=False,
                   enable_asserts=False, num_devices=N_CORES)

    x_d = nc.declare_dram_parameter("x", [n, F], F32, isOutput=False)
    W_d, WA_d, BB_d, CB_d = {}, {}, {}, {}
    for l in (0, 1):
        WA_d[l] = nc.declare_dram_parameter(f"WA_{l}", [F, 34], F32, isOutput=False)
        for h in (0, 1):
            W_d[l, h] = nc.declare_dram_parameter(f"W_{l}_{h}", [F, D], F32, isOutput=False)
            BB_d[l, h] = nc.declare_dram_parameter(f"BB_{l}_{h}", [D], F32, isOutput=False)
            CB_d[l, h] = nc.declare_dram_parameter(f"CB_{l}_{h}", [1], F32, isOutput=False)
    out_d = nc.declare_dram_parameter("out", [n, F], F32, isOutput=True)

    with tile.TileContext(nc) as tc, ExitStack() as ctx:
        const = ctx.enter_context(tc.tile_pool(name="const", bufs=1))
        persist = ctx.enter_context(tc.tile_pool(name="persist", bufs=1))
        headp = ctx.enter_context(tc.tile_pool(name="headp", bufs=2))
        up = ctx.enter_context(tc.tile_pool(name="up", bufs=4))
        t1p = ctx.enter_context(tc.tile_pool(name="t1p", bufs=3))
        epp = ctx.enter_context(tc.tile_pool(name="epp", bufs=2))
        smallp = ctx.enter_context(tc.tile_pool(name="smallp", bufs=4))
        ps_prep = ctx.enter_context(tc.tile_pool(name="ps_prep", bufs=2, space="PSUM"))
        ps_main = ctx.enter_context(tc.tile_pool(name="ps_main", bufs=2, space="PSUM"))
        ps_z = ctx.enter_context(tc.tile_pool(name="ps_z", bufs=1, space="PSUM"))

        # ---- constants ----
        I128 = const.tile([P, P], F32, tag="I128", name="I128")
        make_identity(nc, I128[:])
        ones2f = const.tile([P, 2], F32, tag="ones2f", name="ones2f")
        nc.vector.memset(ones2f[:], 1.0)
        ones2 = const.tile([P, 2], BF16, tag="ones2", name="ones2")
        nc.vector.tensor_copy(ones2[:], ones2f[:])
        # ones row [1, P] f32r: stationary of the PE row-broadcast matmul
        onesrf = const.tile([1, P], F32, tag="onesrf", name="onesrf")
        nc.vector.memset(onesrf[:], 1.0)
        onesr = const.tile([1, P], F32R, tag="onesr", name="onesr")
        nc.vector.tensor_copy(onesr[:], onesrf[:])

        # ---- parameters ----
        Wt, WAt, bt, cbt = {}, {}, {}, {}
        for l in (0, 1):
            WAt[l] = []
            for c in range(2):
                waf = smallp.tile([P, 34], F32, tag="waload", name="waload")
                nc.sync.dma_start(out=waf[:], in_=WA_d[l][c * P:(c + 1) * P, :])
                wa = const.tile([P, 34], F32R, tag=f"WA{l}{c}", name=f"WA{l}{c}")
                nc.vector.tensor_copy(wa[:], waf[:])
                WAt[l].append(wa)
            for h in (0, 1):
                Wt[l, h] = []
                for c in range(2):
                    wf = smallp.tile([P, D], F32, tag="wload", name="wload")
                    nc.sync.dma_start(out=wf[:], in_=W_d[l, h][c * P:(c + 1) * P, :])
                    w = const.tile([P, D], F32R, tag=f"W{l}{h}{c}", name=f"W{l}{h}{c}")
                    nc.vector.tensor_copy(w[:], wf[:])
                    Wt[l, h].append(w)
                b = const.tile([P, 1], F32, tag=f"b{l}{h}", name=f"b{l}{h}")
                nc.sync.dma_start(
                    out=b[:], in_=BB_d[l, h][:].rearrange("(p o) -> p o", o=1))
                bt[l, h] = b
                cb = const.tile([P, 1], F32, tag=f"cb{l}{h}", name=f"cb{l}{h}")
                nc.sync.dma_start(
                    out=cb[:],
                    in_=CB_d[l, h][:].rearrange("(o q) -> o q", o=1).to_broadcast([P, 1]))
                cbt[l, h] = cb

        # ---- load x and transpose to XT [2 x (P, n)] f32r ----
        XT = [persist.tile([P, n], F32R, tag=f"XT{f}", name=f"XT{f}") for f in range(2)]
        for c in range(NJ):
            xc = smallp.tile([P, F], F32, tag="xload", name="xload")
            nc.sync.dma_start(out=xc[:], in_=x_d[c * P:(c + 1) * P, :])
            for f in range(2):
                tp = ps_prep.tile([P, IB], F32, tag="prep", name="prep")
                nc.tensor.transpose(tp[:, 0:P], xc[:, f * P:(f + 1) * P], I128[:])
                if (c + f) % 2 == 0:
                    nc.vector.tensor_copy(XT[f][:, c * P:(c + 1) * P], tp[:, 0:P])
                else:
                    nc.scalar.activation(XT[f][:, c * P:(c + 1) * P], tp[:, 0:P], AF.Copy)

        X1T = [persist.tile([P, n], F32R, tag=f"X1T{f}", name=f"X1T{f}") for f in range(2)]
        X2T = [persist.tile([P, n], F32, tag=f"X2T{f}", name=f"X2T{f}") for f in range(2)]

        def gat_head(XTin, hl_row, hr_row, Wc, bcol, cbcol, OUT):
            # ---- per-head score vectors ----
            # hr in column layout [P, NJ] via small DMAs, + const bias
            hr_raw = headp.tile([P, NJ], F32, tag="hr_raw", name="hr_raw")
            for jc in range(NJ):
                nc.sync.dma_start(out=hr_raw[:, jc:jc + 1],
                                  in_=hr_row[0:1, jc * P:(jc + 1) * P].bitcast(F32))
            hrc = headp.tile([P, NJ], F32, tag="hrc", name="hrc")
            nc.scalar.activation(hrc[:], hr_raw[:], AF.Identity, bias=cbcol[:])
            Bc = headp.tile([P, NJ], F32, tag="Bc", name="Bc")
            nc.scalar.activation(Bc[:], hrc[:], AF.Exp)
            Dc = headp.tile([P, NJ], F32, tag="Dc", name="Dc")
            nc.scalar.activation(Dc[:], hrc[:], AF.Exp, scale=ALPHA)
            # hl broadcast via PE ones-matmul, then exp'd broadcasts on ACT
            hlb = headp.tile([P, n], F32, tag="hlb", name="hlb")
            Ab = headp.tile([P, n], BF16, tag="Ab", name="Ab")
            Cb = headp.tile([P, n], BF16, tag="Cb", name="Cb")
            for q in range(n // IB):
                sl = slice(q * IB, (q + 1) * IB)
                ps = ps_prep.tile([P, IB], F32, tag="prep", name="prep")
                nc.tensor.matmul(ps[:], onesr[:], hl_row[0:1, sl], start=True, stop=True)
                if q % 2 == 0:
                    nc.vector.tensor_copy(hlb[:, sl], ps[:])
                else:
                    nc.scalar.activation(hlb[:, sl], ps[:], AF.Copy)
                nc.scalar.activation(Ab[:, sl], ps[:], AF.Exp)
                nc.scalar.activation(Cb[:, sl], ps[:], AF.Exp, scale=ALPHA)

            # ---- hT = W.T @ XTin + b  [P, n] f32 ----
            hT = headp.tile([P, n], F32, tag="hT", name="hT")
            for q in range(n // IB):
                sl = slice(q * IB, (q + 1) * IB)
                ps = ps_prep.tile([P, IB], F32, tag="prep", name="prep")
                nc.tensor.matmul(ps[:], Wc[0][:], XTin[0][:, sl], start=True, stop=False)
                nc.tensor.matmul(ps[:], Wc[1][:], XTin[1][:, sl], start=False, stop=True)
                nc.scalar.activation(hT[:, sl], ps[:], AF.Identity, bias=bcol[:])
            # ---- h chunks [j, d] bf16 via PE transpose ----
            h_bf = []
            for g in range(4):
                tp = ps_prep.tile([P, IB], F32, tag="prep", name="prep")
                for t in range(4):
                    jc = 4 * g + t
                    nc.tensor.transpose(tp[:, t * P:(t + 1) * P],
                                        hT[:, jc * P:(jc + 1) * P], I128[:])
                hg = headp.tile([P, IB], BF16, tag=f"hbf{g}", name=f"hbf{g}")
                nc.vector.tensor_copy(hg[:], tp[:])
                h_bf.append(hg)

            # ---- attention ----
            for half in range(NH):
                i0 = half * HW
                isl = slice(i0, i0 + HW)
                oacc = [ps_main.tile([P, IB], F32, tag=f"oacc{k}", name=f"oacc{k}")
                        for k in range(KH)]
                zacc = [ps_z.tile([2, IB], F32, tag=f"zacc{k}", name=f"zacc{k}")
                        for k in range(KH)]
                for jc in range(NJ):
                    u = up.tile([P, HW], BF16, tag="u", name="u")
                    if jc in ACT_T1:
                        t1 = t1p.tile([P, HW], BF16, tag="t1", name="t1")
                        nc.scalar.activation(t1[:], hlb[:, isl], AF.Exp,
                                             bias=hrc[:, jc:jc + 1])
                    else:
                        t1 = t1p.tile([P, HW], BF16, tag="t1", name="t1")
                        nc.vector.tensor_scalar(t1[:], Ab[:, isl],
                                                Bc[:, jc:jc + 1], None,
                                                AluOpType.mult)
                    nc.vector.scalar_tensor_tensor(
                        u[:], in0=Cb[:, isl], scalar=Dc[:, jc:jc + 1], in1=t1[:],
                        op0=AluOpType.mult, op1=AluOpType.max)
                    # zero the global diagonal (adjacency excludes self; the
                    # +identity is added post-softmax via v2 = v + hT)
                    dcol = jc * P
                    if i0 <= dcol < i0 + HW:
                        lo = dcol - i0
                        nc.gpsimd.affine_select(
                            out=u[:, lo:lo + P], in_=u[:, lo:lo + P],
                            compare_op=AluOpType.not_equal, fill=0.0,
                            base=0, pattern=[[-1, P]], channel_multiplier=1)
                    for k in range(KH):
                        nc.tensor.matmul(oacc[k][:], h_bf[jc // 4][:, (jc % 4) * P:(jc % 4 + 1) * P],
                                         u[:, k * IB:(k + 1) * IB],
                                         start=(jc == 0), stop=(jc == NJ - 1))
                    for k in range(KH):
                        nc.tensor.matmul(zacc[k][:], ones2[:],
                                         u[:, k * IB:(k + 1) * IB],
                                         start=(jc == 0), stop=(jc == NJ - 1))
                # ---- epilogue for this half ----
                zrow = epp.tile([1, HW], F32, tag="zrow", name="zrow")
                for k in range(KH):
                    nc.vector.tensor_copy(zrow[0:1, k * IB:(k + 1) * IB],
                                          zacc[k][0:1, :])
                zcol = epp.tile([P, HW // P], F32, tag="zcol", name="zcol")
                for q in range(HW // P):
                    nc.sync.dma_start(out=zcol[:, q:q + 1],
                                      in_=zrow[0:1, q * P:(q + 1) * P])
                rcol = epp.tile([P, HW // P], F32, tag="rcol", name="rcol")
                nc.vector.reciprocal(rcol[:], zcol[:])
                rrow = epp.tile([1, HW], F32, tag="rrow", name="rrow")
                for q in range(HW // P):
                    nc.sync.dma_start(out=rrow[0:1, q * P:(q + 1) * P],
                                      in_=rcol[:, q:q + 1])
                for k in range(KH):
                    ksl = slice(i0 + k * IB, i0 + (k + 1) * IB)
                    rb = ps_prep.tile([P, IB], F32, tag="prep", name="prep")
                    nc.tensor.matmul(rb[:], onesr[:],
                                     rrow[0:1, k * IB:(k + 1) * IB].bitcast(F32R),
                                     start=True, stop=True)
                    ob = epp.tile([P, IB], F32, tag="ob", name="ob")
                    nc.scalar.activation(ob[:], oacc[k][:], AF.Copy)
                    v = epp.tile([P, IB], F32, tag="v", name="v")
                    nc.vector.tensor_tensor(v[:], ob[:], rb[:], AluOpType.mult)
                    v2 = epp.tile([P, IB], F32, tag="v2", name="v2")
                    nc.vector.tensor_tensor(v2[:], v[:], hT[:, ksl], AluOpType.add)
                    # y = elu(v2) + 1 = max(v2, 0) + exp(min(v2, 0))
                    m = epp.tile([P, IB], F32, tag="m", name="m")
                    nc.gpsimd.tensor_scalar_min(m[:], v2[:], 0.0)
                    e = epp.tile([P, IB], F32, tag="e", name="e")
                    nc.scalar.activation(e[:], m[:], AF.Exp)
                    nc.vector.scalar_tensor_tensor(
                        OUT[:, ksl], in0=v2[:], scalar=0.0, in1=e[:],
                        op0=AluOpType.max, op1=AluOpType.add)

        def gat_layer(XTin, l, XTout):
            # score rows [4, n]: (hl_h0, hr_h0, hl_h1, hr_h1)
            rows = [headp.tile([2, n], F32R, tag=f"rows{h}", name=f"rows{h}")
                    for h in (0, 1)]
            for q in range(n // IB):
                sl = slice(q * IB, (q + 1) * IB)
                ps = ps_prep.tile([34, IB], F32, tag="prep", name="prep")
                nc.tensor.matmul(ps[:], WAt[l][0][:], XTin[0][:, sl], start=True, stop=False)
                nc.tensor.matmul(ps[:], WAt[l][1][:], XTin[1][:, sl], start=False, stop=True)
                nc.vector.tensor_copy(rows[0][:, sl], ps[0:2, :])
                nc.vector.tensor_copy(rows[1][:, sl], ps[32:34, :])
            for h in (0, 1):
                gat_head(XTin, rows[h][0:1, :], rows[h][1:2, :],
                         Wt[l, h], bt[l, h], cbt[l, h], XTout[h])

        gat_layer(XT, 0, X1T)
        gat_layer(X1T, 1, X2T)

        # ---- transpose X2T back, subtract the elu+1 carry, store ----
        for c in range(NJ):
            ob = smallp.tile([P, F], F32, tag="ost", name="ost")
            for f in range(2):
                tp = ps_prep.tile([P, IB], F32, tag="prep", name="prep")
                nc.tensor.transpose(tp[:, 0:P], X2T[f][:, c * P:(c + 1) * P], I128[:])
                if (c + f) % 2 == 0:
                    nc.vector.tensor_scalar_add(ob[:, f * P:(f + 1) * P], tp[:, 0:P], -1.0)
                else:
                    nc.scalar.activation(ob[:, f * P:(f + 1) * P], tp[:, 0:P],
                                         AF.Copy, bias=-1.0)
            nc.sync.dma_start(out=out_d[c * P:(c + 1) * P, :], in_=ob[:])

    nc.compile()
    return nc


_CACHE = {}
LAST_RESULTS = None


def kernel(**inputs):
    global LAST_RESULTS
    from concourse.bass_utils import run_bass_kernel_spmd

    x = np.ascontiguousarray(np.asarray(inputs["x"], dtype=np.float32))
    B = x.shape[0]
    assert B == N_CORES and x.shape[1] == N and x.shape[2] == F

    if "nc" not in _CACHE:
        _CACHE["nc"] = build_nc()
    nc = _CACHE["nc"]

    base = {}
    for l in (0, 1):
        wa_cols = []
        for h in (0, 1):
            W = np.asarray(inputs[f"W_{l}_{h}"], dtype=np.float64)
            b = np.asarray(inputs[f"b_{l}_{h}"], dtype=np.float64)
            a = np.asarray(inputs[f"a_{l}_{h}"], dtype=np.float64).reshape(-1)
            wa_l = W @ a[:D]
            wa_r = W @ a[D:]
            wa_cols.extend([wa_l, wa_r])
            cb = float(b @ a[:D] + b @ a[D:])
            bb = b.copy()
            if l == 1:
                # layer-2 inputs carry elu+1: x2 = y - 1
                bb = b - W.sum(axis=0)
                cb = cb - float(wa_l.sum() + wa_r.sum())
            base[f"W_{l}_{h}"] = np.ascontiguousarray(W.astype(np.float32))
            base[f"BB_{l}_{h}"] = np.ascontiguousarray(bb.astype(np.float32))
            base[f"CB_{l}_{h}"] = np.array([cb], dtype=np.float32)
        wa_pad = np.zeros((F, 34), dtype=np.float64)
        wa_pad[:, 0] = wa_cols[0]
        wa_pad[:, 1] = wa_cols[1]
        wa_pad[:, 32] = wa_cols[2]
        wa_pad[:, 33] = wa_cols[3]
        base[f"WA_{l}"] = np.ascontiguousarray(wa_pad.astype(np.float32))

    in_maps = [dict(base, x=np.ascontiguousarray(x[i])) for i in range(B)]
    res = run_bass_kernel_spmd(nc, in_maps, list(range(N_CORES)),
                               trace=bool(os.environ.get("BASS_TRACE")))
    LAST_RESULTS = res
    out = np.stack([res.results[i]["out"] for i in range(B)], axis=0)
    return out.astype(np.float32)


# revision 21
# speedup vs baseline: 1.3059x; 1.3059x over previous
"""Dense 2-layer 2-head GAT for Trainium2 (Bass/Tile), data-parallel over batch.

v2 — rank-1 score factorization. Per head the score matrix is
s[i,j] = lrelu(hl_i + hr_j), so

  u = exp(lrelu(s)) = max(exp(s), exp(0.2 s))
    = max(outer(e^{hr}, e^{hl}), outer(e^{0.2 hr}, e^{0.2 hl}))

i.e. an elementwise max of two rank-1 outer products.  hl/hr come from one
tiny [4, n] matmul per layer with host-folded wa = W @ a, so score-tile
generation needs only TWO elementwise passes per tile:

  pass A:  t1 = e^{s}  — either ACT Exp(hl_bcast + hr_j bias)  (per-partition
           bias), or DVE tensor_scalar A_bcast * B_j (4x bf16 mode);
           the per-jc assignment is static to balance ACT vs DVE.
  pass B:  u = DVE stt (C_bcast * D_j) max t1   (2x bf16 mode)

Row-vector broadcasts (hl, A=e^hl, C=e^{0.2 hl}, 1/Z) are materialized by
DMA (idle engine) instead of PE broadcast matmuls.  The diagonal mask is
applied at the source (u diag = 0) by gpsimd affine_select, so no
numerator/denominator correction pass is needed; softmax's +identity is the
v2 = v + hT add.  elu is carried as elu+1 (saves a pass); layer-2 params are
host-adjusted (b2' = b2 - colsum(W2), score-bias consts) and the final store
subtracts 1 during the transpose copy-out.

Z (softmax denominator) is a [128,2]-stationary bf16 ones-matmul on the PE;
u and the h stationary operand are bf16 (scores themselves stay fp32).
"""

import os
from contextlib import ExitStack

import numpy as np

import concourse.bass as bass
import concourse.mybir as mybir
import concourse.tile as tile
from concourse.alu_op_type import AluOpType
from concourse.masks import make_identity

F32 = mybir.dt.float32
F32R = mybir.dt.float32r
BF16 = mybir.dt.bfloat16
AF = mybir.ActivationFunctionType

N = 2048
F = 256
D = 128
P = 128
ALPHA = 0.2
N_CORES = 8

NJ = N // P          # 16 j-chunks
IB = 512             # PSUM bank free width (fp32)
HW = 1024            # i-half width
NH = N // HW         # 2 halves
KH = HW // IB        # 2 k-blocks per half

# jc tiles whose t2 branch is computed on ACT (Exp scale=0.2 w/ bias) and
# combined with a DVE tensor_tensor max (2x bf16); the rest use the DVE
# scalar_tensor_tensor (1x) path.  Tuned to balance ACT vs DVE load.
T2_ACT = (0, 3, 6, 9, 12)


def build_nc(n=N):
    from concourse import bacc
    nc = bacc.Bacc("TRN2", target_bir_lowering=False, debug=False,
                   enable_asserts=False, num_devices=N_CORES)

    x_d = nc.declare_dram_parameter("x", [n, F], F32, isOutput=False)
    W_d, WA_d, BB_d, CB_d = {}, {}, {}, {}
    for l in (0, 1):
        WA_d[l] = nc.declare_dram_parameter(f"WA_{l}", [F, 34], F32, isOutput=False)
        for h in (0, 1):
            W_d[l, h] = nc.declare_dram_parameter(f"W_{l}_{h}", [F, D], F32, isOutput=False)
            BB_d[l, h] = nc.declare_dram_parameter(f"BB_{l}_{h}", [D], F32, isOutput=False)
            CB_d[l, h] = nc.declare_dram_parameter(f"CB_{l}_{h}", [1], F32, isOutput=False)
    out_d = nc.declare_dram_parameter("out", [n, F], F32, isOutput=True)

    with tile.TileContext(nc) as tc, ExitStack() as ctx:
        const = ctx.enter_context(tc.tile_pool(name="const", bufs=1))
        persist = ctx.enter_context(tc.tile_pool(name="persist", bufs=1))
        headp = ctx.enter_context(tc.tile_pool(name="headp", bufs=2))
        up = ctx.enter_context(tc.tile_pool(name="up", bufs=4))
        t1p = ctx.enter_context(tc.tile_pool(name="t1p", bufs=3))
        epp = ctx.enter_context(tc.tile_pool(name="epp", bufs=2))
        smallp = ctx.enter_context(tc.tile_pool(name="smallp", bufs=4))
        ps_prep = ctx.enter_context(tc.tile_pool(name="ps_prep", bufs=2, space="PSUM"))
        ps_main = ctx.enter_context(tc.tile_pool(name="ps_main", bufs=2, space="PSUM"))
        ps_z = ctx.enter_context(tc.tile_pool(name="ps_z", bufs=1, space="PSUM"))

        # ---- constants ----
        I128 = const.tile([P, P], F32, tag="I128", name="I128")
        make_identity(nc, I128[:])
        ones2f = const.tile([P, 2], F32, tag="ones2f", name="ones2f")
        nc.vector.memset(ones2f[:], 1.0)
        ones2 = const.tile([P, 2], BF16, tag="ones2", name="ones2")
        nc.vector.tensor_copy(ones2[:], ones2f[:])
        # ones row [1, P] f32r: stationary of the PE row-broadcast matmul
        onesrf = const.tile([1, P], F32, tag="onesrf", name="onesrf")
        nc.vector.memset(onesrf[:], 1.0)
        onesr = const.tile([1, P], F32R, tag="onesr", name="onesr")
        nc.vector.tensor_copy(onesr[:], onesrf[:])

        # ---- parameters ----
        Wt, WAt, bt, cbt = {}, {}, {}, {}
        for l in (0, 1):
            WAt[l] = []
            for c in range(2):
                waf = smallp.tile([P, 34], F32, tag="waload", name="waload")
                nc.sync.dma_start(out=waf[:], in_=WA_d[l][c * P:(c + 1) * P, :])
                wa = const.tile([P, 34], F32R, tag=f"WA{l}{c}", name=f"WA{l}{c}")
                nc.vector.tensor_copy(wa[:], waf[:])
                WAt[l].append(wa)
            for h in (0, 1):
                Wt[l, h] = []
                for c in range(2):
                    wf = smallp.tile([P, D], F32, tag="wload", name="wload")
                    nc.sync.dma_start(out=wf[:], in_=W_d[l, h][c * P:(c + 1) * P, :])
                    w = const.tile([P, D], F32R, tag=f"W{l}{h}{c}", name=f"W{l}{h}{c}")
                    nc.vector.tensor_copy(w[:], wf[:])
                    Wt[l, h].append(w)
                b = const.tile([P, 1], F32, tag=f"b{l}{h}", name=f"b{l}{h}")
                nc.sync.dma_start(
                    out=b[:], in_=BB_d[l, h][:].rearrange("(p o) -> p o", o=1))
                bt[l, h] = b
                cb = const.tile([P, 1], F32, tag=f"cb{l}{h}", name=f"cb{l}{h}")
                nc.sync.dma_start(
                    out=cb[:],
                    in_=CB_d[l, h][:].rearrange("(o q) -> o q", o=1).to_broadcast([P, 1]))
                cbt[l, h] = cb

        # ---- load x and transpose to XT [2 x (P, n)] f32r ----
        XT = [persist.tile([P, n], F32R, tag=f"XT{f}", name=f"XT{f}") for f in range(2)]
        for c in range(NJ):
            xc = smallp.tile([P, F], F32, tag="xload", name="xload")
            nc.sync.dma_start(out=xc[:], in_=x_d[c * P:(c + 1) * P, :])
            for f in range(2):
                tp = ps_prep.tile([P, IB], F32, tag="prep", name="prep")
                nc.tensor.transpose(tp[:, 0:P], xc[:, f * P:(f + 1) * P], I128[:])
                if (c + f) % 2 == 0:
                    nc.vector.tensor_copy(XT[f][:, c * P:(c + 1) * P], tp[:, 0:P])
                else:
                    nc.scalar.activation(XT[f][:, c * P:(c + 1) * P], tp[:, 0:P], AF.Copy)

        X1T = [persist.tile([P, n], F32R, tag=f"X1T{f}", name=f"X1T{f}") for f in range(2)]
        X2T = [persist.tile([P, n], F32, tag=f"X2T{f}", name=f"X2T{f}") for f in range(2)]

        def gat_head(XTin, hl_row, hr_row, Wc, bcol, cbcol, OUT):
            # ---- per-head score vectors ----
            # hr in column layout [P, NJ] via small DMAs, + const bias
            hr_raw = headp.tile([P, NJ], F32, tag="hr_raw", name="hr_raw")
            for jc in range(NJ):
                nc.sync.dma_start(out=hr_raw[:, jc:jc + 1],
                                  in_=hr_row[0:1, jc * P:(jc + 1) * P].bitcast(F32))
            hrc = headp.tile([P, NJ], F32, tag="hrc", name="hrc")
            nc.scalar.activation(hrc[:], hr_raw[:], AF.Identity, bias=cbcol[:])
            Dc = headp.tile([P, NJ], F32, tag="Dc", name="Dc")
            nc.scalar.activation(Dc[:], hrc[:], AF.Exp, scale=ALPHA)
            hrc02 = headp.tile([P, NJ], F32, tag="hrc02", name="hrc02")
            nc.scalar.activation(hrc02[:], hrc[:], AF.Identity, scale=ALPHA)
            # hl broadcast via PE ones-matmul, then exp'd broadcast on ACT
            hlb = headp.tile([P, n], F32, tag="hlb", name="hlb")
            Cb = headp.tile([P, n], BF16, tag="Cb", name="Cb")
            for q in range(n // IB):
                sl = slice(q * IB, (q + 1) * IB)
                ps = ps_prep.tile([P, IB], F32, tag="prep", name="prep")
                nc.tensor.matmul(ps[:], onesr[:], hl_row[0:1, sl], start=True, stop=True)
                nc.vector.tensor_copy(hlb[:, sl], ps[:])
                nc.scalar.activation(Cb[:, sl], ps[:], AF.Exp, scale=ALPHA)

            # ---- hT = W.T @ XTin + b  [P, n] f32 ----
            hT = headp.tile([P, n], F32, tag="hT", name="hT")
            for q in range(n // IB):
                sl = slice(q * IB, (q + 1) * IB)
                ps = ps_prep.tile([P, IB], F32, tag="prep", name="prep")
                nc.tensor.matmul(ps[:], Wc[0][:], XTin[0][:, sl], start=True, stop=False)
                nc.tensor.matmul(ps[:], Wc[1][:], XTin[1][:, sl], start=False, stop=True)
                nc.scalar.activation(hT[:, sl], ps[:], AF.Identity, bias=bcol[:])
            # ---- h chunks [j, d] bf16 via PE transpose ----
            h_bf = []
            for g in range(4):
                tp = ps_prep.tile([P, IB], F32, tag="prep", name="prep")
                for t in range(4):
                    jc = 4 * g + t
                    nc.tensor.transpose(tp[:, t * P:(t + 1) * P],
                                        hT[:, jc * P:(jc + 1) * P], I128[:])
                hg = headp.tile([P, IB], BF16, tag=f"hbf{g}", name=f"hbf{g}")
                nc.vector.tensor_copy(hg[:], tp[:])
                h_bf.append(hg)

            # ---- attention ----
            for half in range(NH):
                i0 = half * HW
                isl = slice(i0, i0 + HW)
                oacc = [ps_main.tile([P, IB], F32, tag=f"oacc{k}", name=f"oacc{k}")
                        for k in range(KH)]
                zacc = [ps_z.tile([2, IB], F32, tag=f"zacc{k}", name=f"zacc{k}")
                        for k in range(KH)]
                for jc in range(NJ):
                    u = up.tile([P, HW], BF16, tag="u", name="u")
                    t1 = t1p.tile([P, HW], BF16, tag="t1", name="t1")
                    nc.scalar.activation(t1[:], hlb[:, isl], AF.Exp,
                                         bias=hrc[:, jc:jc + 1])
                    if jc in T2_ACT:
                        t2 = t1p.tile([P, HW], BF16, tag="t2", name="t2")
                        nc.scalar.activation(t2[:], hlb[:, isl], AF.Exp,
                                             scale=ALPHA, bias=hrc02[:, jc:jc + 1])
                        nc.vector.tensor_tensor(u[:], t1[:], t2[:], AluOpType.max)
                    else:
                        nc.vector.scalar_tensor_tensor(
                            u[:], in0=Cb[:, isl], scalar=Dc[:, jc:jc + 1], in1=t1[:],
                            op0=AluOpType.mult, op1=AluOpType.max)
                    # zero the global diagonal (adjacency excludes self; the
                    # +identity is added post-softmax via v2 = v + hT)
                    dcol = jc * P
                    if i0 <= dcol < i0 + HW:
                        lo = dcol - i0
                        nc.gpsimd.affine_select(
                            out=u[:, lo:lo + P], in_=u[:, lo:lo + P],
                            compare_op=AluOpType.not_equal, fill=0.0,
                            base=0, pattern=[[-1, P]], channel_multiplier=1)
                    for k in range(KH):
                        nc.tensor.matmul(oacc[k][:], h_bf[jc // 4][:, (jc % 4) * P:(jc % 4 + 1) * P],
                                         u[:, k * IB:(k + 1) * IB],
                                         start=(jc == 0), stop=(jc == NJ - 1))
                    for k in range(KH):
                        nc.tensor.matmul(zacc[k][:], ones2[:],
                                         u[:, k * IB:(k + 1) * IB],
                                         start=(jc == 0), stop=(jc == NJ - 1))
                # ---- epilogue for this half ----
                zrow = epp.tile([1, HW], F32, tag="zrow", name="zrow")
                for k in range(KH):
                    nc.vector.tensor_copy(zrow[0:1, k * IB:(k + 1) * IB],
                                          zacc[k][0:1, :])
                zcol = epp.tile([P, HW // P], F32, tag="zcol", name="zcol")
                for q in range(HW // P):
                    nc.sync.dma_start(out=zcol[:, q:q + 1],
                                      in_=zrow[0:1, q * P:(q + 1) * P])
                rcol = epp.tile([P, HW // P], F32, tag="rcol", name="rcol")
                nc.vector.reciprocal(rcol[:], zcol[:])
                rrow = epp.tile([1, HW], F32, tag="rrow", name="rrow")
                for q in range(HW // P):
                    nc.sync.dma_start(out=rrow[0:1, q * P:(q + 1) * P],
                                      in_=rcol[:, q:q + 1])
                for k in range(KH):
                    ksl = slice(i0 + k * IB, i0 + (k + 1) * IB)
                    rb = ps_prep.tile([P, IB], F32, tag="prep", name="prep")
                    nc.tensor.matmul(rb[:], onesr[:],
                                     rrow[0:1, k * IB:(k + 1) * IB].bitcast(F32R),
                                     start=True, stop=True)
                    ob = epp.tile([P, IB], F32, tag="ob", name="ob")
                    nc.scalar.activation(ob[:], oacc[k][:], AF.Copy)
                    v = epp.tile([P, IB], F32, tag="v", name="v")
                    nc.vector.tensor_tensor(v[:], ob[:], rb[:], AluOpType.mult)
                    v2 = epp.tile([P, IB], F32, tag="v2", name="v2")
                    nc.vector.tensor_tensor(v2[:], v[:], hT[:, ksl], AluOpType.add)
                    # y = elu(v2) + 1 = max(v2, 0) + exp(min(v2, 0))
                    m = epp.tile([P, IB], F32, tag="m", name="m")
                    nc.vector.tensor_scalar_min(m[:], v2[:], 0.0)
                    e = epp.tile([P, IB], F32, tag="e", name="e")
                    nc.scalar.activation(e[:], m[:], AF.Exp)
                    nc.vector.scalar_tensor_tensor(
                        OUT[:, ksl], in0=v2[:], scalar=0.0, in1=e[:],
                        op0=AluOpType.max, op1=AluOpType.add)

        def gat_layer(XTin, l, XTout):
            # score rows [4, n]: (hl_h0, hr_h0, hl_h1, hr_h1)
            rows = [headp.tile([2, n], F32R, tag=f"rows{h}", name=f"rows{h}")
                    for h in (0, 1)]
            for q in range(n // IB):
                sl = slice(q * IB, (q + 1) * IB)
                ps = ps_prep.tile([34, IB], F32, tag="prep", name="prep")
                nc.tensor.matmul(ps[:], WAt[l][0][:], XTin[0][:, sl], start=True, stop=False)
                nc.tensor.matmul(ps[:], WAt[l][1][:], XTin[1][:, sl], start=False, stop=True)
                nc.vector.tensor_copy(rows[0][:, sl], ps[0:2, :])
                nc.vector.tensor_copy(rows[1][:, sl], ps[32:34, :])
            for h in (0, 1):
                gat_head(XTin, rows[h][0:1, :], rows[h][1:2, :],
                         Wt[l, h], bt[l, h], cbt[l, h], XTout[h])

        gat_layer(XT, 0, X1T)
        gat_layer(X1T, 1, X2T)

        # ---- transpose X2T back, subtract the elu+1 carry, store ----
        for c in range(NJ):
            ob = smallp.tile([P, F], F32, tag="ost", name="ost")
            for f in range(2):
                tp = ps_prep.tile([P, IB], F32, tag="prep", name="prep")
                nc.tensor.transpose(tp[:, 0:P], X2T[f][:, c * P:(c + 1) * P], I128[:])
                if (c + f) % 2 == 0:
                    nc.vector.tensor_scalar_add(ob[:, f * P:(f + 1) * P], tp[:, 0:P], -1.0)
                else:
                    nc.scalar.activation(ob[:, f * P:(f + 1) * P], tp[:, 0:P],
                                         AF.Copy, bias=-1.0)
            nc.sync.dma_start(out=out_d[c * P:(c + 1) * P, :], in_=ob[:])

    nc.compile()
    return nc


_CACHE = {}
LAST_RESULTS = None


def kernel(**inputs):
    global LAST_RESULTS
    from concourse.bass_utils import run_bass_kernel_spmd

    x = np.ascontiguousarray(np.asarray(inputs["x"], dtype=np.float32))
    B = x.shape[0]
    assert B == N_CORES and x.shape[1] == N and x.shape[2] == F

    if "nc" not in _CACHE:
        _CACHE["nc"] = build_nc()
    nc = _CACHE["nc"]

    base = {}
    for l in (0, 1):
        wa_cols = []
        for h in (0, 1):
            W = np.asarray(inputs[f"W_{l}_{h}"], dtype=np.float64)
            b = np.asarray(inputs[f"b_{l}_{h}"], dtype=np.float64)
            a = np.asarray(inputs[f"a_{l}_{h}"], dtype=np.float64).reshape(-1)
            wa_l = W @ a[:D]
            wa_r = W @ a[D:]
            wa_cols.extend([wa_l, wa_r])
            cb = float(b @ a[:D] + b @ a[D:])
            bb = b.copy()
            if l == 1:
                # layer-2 inputs carry elu+1: x2 = y - 1
                bb = b - W.sum(axis=0)
                cb = cb - float(wa_l.sum() + wa_r.sum())
            base[f"W_{l}_{h}"] = np.ascontiguousarray(W.astype(np.float32))
            base[f"BB_{l}_{h}"] = np.ascontiguousarray(bb.astype(np.float32))
            base[f"CB_{l}_{h}"] = np.array([cb], dtype=np.float32)
        wa_pad = np.zeros((F, 34), dtype=np.float64)
        wa_pad[:, 0] = wa_cols[0]
        wa_pad[:, 1] = wa_cols[1]
        wa_pad[:, 32] = wa_cols[2]
        wa_pad[:, 33] = wa_cols[3]
        base[f"WA_{l}"] = np.ascontiguousarray(wa_pad.astype(np.float32))

    in_maps = [dict(base, x=np.ascontiguousarray(x[i])) for i in range(B)]
    res = run_bass_kernel_spmd(nc, in_maps, list(range(N_CORES)),
                               trace=bool(os.environ.get("BASS_TRACE")))
    LAST_RESULTS = res
    out = np.stack([res.results[i]["out"] for i in range(B)], axis=0)
    return out.astype(np.float32)


# revision 25
# speedup vs baseline: 1.3117x; 1.0044x over previous
"""Dense 2-layer 2-head GAT for Trainium2 (Bass/Tile), data-parallel over batch.

v2 — rank-1 score factorization. Per head the score matrix is
s[i,j] = lrelu(hl_i + hr_j), so

  u = exp(lrelu(s)) = max(exp(s), exp(0.2 s))
    = max(outer(e^{hr}, e^{hl}), outer(e^{0.2 hr}, e^{0.2 hl}))

i.e. an elementwise max of two rank-1 outer products.  hl/hr come from one
tiny [4, n] matmul per layer with host-folded wa = W @ a, so score-tile
generation needs only TWO elementwise passes per tile:

  pass A:  t1 = e^{s}  — either ACT Exp(hl_bcast + hr_j bias)  (per-partition
           bias), or DVE tensor_scalar A_bcast * B_j (4x bf16 mode);
           the per-jc assignment is static to balance ACT vs DVE.
  pass B:  u = DVE stt (C_bcast * D_j) max t1   (2x bf16 mode)

Row-vector broadcasts (hl, A=e^hl, C=e^{0.2 hl}, 1/Z) are materialized by
DMA (idle engine) instead of PE broadcast matmuls.  The diagonal mask is
applied at the source (u diag = 0) by gpsimd affine_select, so no
numerator/denominator correction pass is needed; softmax's +identity is the
v2 = v + hT add.  elu is carried as elu+1 (saves a pass); layer-2 params are
host-adjusted (b2' = b2 - colsum(W2), score-bias consts) and the final store
subtracts 1 during the transpose copy-out.

Z (softmax denominator) is a [128,2]-stationary bf16 ones-matmul on the PE;
u and the h stationary operand are bf16 (scores themselves stay fp32).
"""

import os
from contextlib import ExitStack

import numpy as np

import concourse.bass as bass
import concourse.mybir as mybir
import concourse.tile as tile
from concourse.alu_op_type import AluOpType
from concourse.masks import make_identity

F32 = mybir.dt.float32
F32R = mybir.dt.float32r
BF16 = mybir.dt.bfloat16
AF = mybir.ActivationFunctionType

N = 2048
F = 256
D = 128
P = 128
ALPHA = 0.2
N_CORES = 8

NJ = N // P          # 16 j-chunks
IB = 512             # PSUM bank free width (fp32)
HW = 1024            # i-half width
NH = N // HW         # 2 halves
KH = HW // IB        # 2 k-blocks per half

# jc tiles whose t2 branch is computed on ACT (Exp scale=0.2 w/ bias) and
# combined with a DVE tensor_tensor max (2x bf16); the rest use the DVE
# scalar_tensor_tensor (1x) path.  Tuned to balance ACT vs DVE load.
T2_ACT = (0, 3, 6, 9, 12)


def build_nc(n=N):
    from concourse import bacc
    nc = bacc.Bacc("TRN2", target_bir_lowering=False, debug=False,
                   enable_asserts=False, num_devices=N_CORES)

    x_d = nc.declare_dram_parameter("x", [n, F], F32, isOutput=False)
    W_d, WA_d, BB_d, CB_d = {}, {}, {}, {}
    for l in (0, 1):
        WA_d[l] = nc.declare_dram_parameter(f"WA_{l}", [F, 34], F32, isOutput=False)
        for h in (0, 1):
            W_d[l, h] = nc.declare_dram_parameter(f"W_{l}_{h}", [F, D], F32, isOutput=False)
            BB_d[l, h] = nc.declare_dram_parameter(f"BB_{l}_{h}", [D], F32, isOutput=False)
            CB_d[l, h] = nc.declare_dram_parameter(f"CB_{l}_{h}", [1], F32, isOutput=False)
    out_d = nc.declare_dram_parameter("out", [n, F], F32, isOutput=True)

    with tile.TileContext(nc) as tc, ExitStack() as ctx:
        const = ctx.enter_context(tc.tile_pool(name="const", bufs=1))
        persist = ctx.enter_context(tc.tile_pool(name="persist", bufs=1))
        headp = ctx.enter_context(tc.tile_pool(name="headp", bufs=2))
        up = ctx.enter_context(tc.tile_pool(name="up", bufs=6))
        t1p = ctx.enter_context(tc.tile_pool(name="t1p", bufs=5))
        epp = ctx.enter_context(tc.tile_pool(name="epp", bufs=2))
        smallp = ctx.enter_context(tc.tile_pool(name="smallp", bufs=4))
        ps_prep = ctx.enter_context(tc.tile_pool(name="ps_prep", bufs=2, space="PSUM"))
        ps_main = ctx.enter_context(tc.tile_pool(name="ps_main", bufs=2, space="PSUM"))
        ps_z = ctx.enter_context(tc.tile_pool(name="ps_z", bufs=1, space="PSUM"))

        # ---- constants ----
        I128 = const.tile([P, P], F32, tag="I128", name="I128")
        make_identity(nc, I128[:])
        ones2f = const.tile([P, 2], F32, tag="ones2f", name="ones2f")
        nc.vector.memset(ones2f[:], 1.0)
        ones2 = const.tile([P, 2], BF16, tag="ones2", name="ones2")
        nc.vector.tensor_copy(ones2[:], ones2f[:])
        # ones row [1, P] f32r: stationary of the PE row-broadcast matmul
        onesrf = const.tile([1, P], F32, tag="onesrf", name="onesrf")
        nc.vector.memset(onesrf[:], 1.0)
        onesr = const.tile([1, P], F32R, tag="onesr", name="onesr")
        nc.vector.tensor_copy(onesr[:], onesrf[:])

        # ---- parameters ----
        Wt, WAt, bt, cbt = {}, {}, {}, {}
        for l in (0, 1):
            WAt[l] = []
            for c in range(2):
                waf = smallp.tile([P, 34], F32, tag="waload", name="waload")
                nc.sync.dma_start(out=waf[:], in_=WA_d[l][c * P:(c + 1) * P, :])
                wa = const.tile([P, 34], F32R, tag=f"WA{l}{c}", name=f"WA{l}{c}")
                nc.vector.tensor_copy(wa[:], waf[:])
                WAt[l].append(wa)
            for h in (0, 1):
                Wt[l, h] = []
                for c in range(2):
                    wf = smallp.tile([P, D], F32, tag="wload", name="wload")
                    nc.sync.dma_start(out=wf[:], in_=W_d[l, h][c * P:(c + 1) * P, :])
                    w = const.tile([P, D], F32R, tag=f"W{l}{h}{c}", name=f"W{l}{h}{c}")
                    nc.vector.tensor_copy(w[:], wf[:])
                    Wt[l, h].append(w)
                b = const.tile([P, 1], F32, tag=f"b{l}{h}", name=f"b{l}{h}")
                nc.sync.dma_start(
                    out=b[:], in_=BB_d[l, h][:].rearrange("(p o) -> p o", o=1))
                bt[l, h] = b
                cb = const.tile([P, 1], F32, tag=f"cb{l}{h}", name=f"cb{l}{h}")
                nc.sync.dma_start(
                    out=cb[:],
                    in_=CB_d[l, h][:].rearrange("(o q) -> o q", o=1).to_broadcast([P, 1]))
                cbt[l, h] = cb

        # ---- load x and transpose to XT [2 x (P, n)] f32r ----
        XT = [persist.tile([P, n], F32R, tag=f"XT{f}", name=f"XT{f}") for f in range(2)]
        for c in range(NJ):
            xc = smallp.tile([P, F], F32, tag="xload", name="xload")
            nc.sync.dma_start(out=xc[:], in_=x_d[c * P:(c + 1) * P, :])
            for f in range(2):
                tp = ps_prep.tile([P, IB], F32, tag="prep", name="prep")
                nc.tensor.transpose(tp[:, 0:P], xc[:, f * P:(f + 1) * P], I128[:])
                if (c + f) % 2 == 0:
                    nc.vector.tensor_copy(XT[f][:, c * P:(c + 1) * P], tp[:, 0:P])
                else:
                    nc.scalar.activation(XT[f][:, c * P:(c + 1) * P], tp[:, 0:P], AF.Copy)

        X1T = [persist.tile([P, n], F32R, tag=f"X1T{f}", name=f"X1T{f}") for f in range(2)]
        X2T = XT  # layer-2 output reuses the x tiles (x is consumed by then)

        def gat_head(XTin, hl_row, hr_row, Wc, bcol, cbcol, OUT):
            # ---- per-head score vectors ----
            # hr in column layout [P, NJ] via small DMAs, + const bias
            hr_raw = headp.tile([P, NJ], F32, tag="hr_raw", name="hr_raw")
            for jc in range(NJ):
                nc.sync.dma_start(out=hr_raw[:, jc:jc + 1],
                                  in_=hr_row[0:1, jc * P:(jc + 1) * P].bitcast(F32))
            hrc = headp.tile([P, NJ], F32, tag="hrc", name="hrc")
            nc.scalar.activation(hrc[:], hr_raw[:], AF.Identity, bias=cbcol[:])
            Dc = headp.tile([P, NJ], F32, tag="Dc", name="Dc")
            nc.scalar.activation(Dc[:], hrc[:], AF.Exp, scale=ALPHA)
            hrc02 = headp.tile([P, NJ], F32, tag="hrc02", name="hrc02")
            nc.scalar.activation(hrc02[:], hrc[:], AF.Identity, scale=ALPHA)
            # hl broadcast via PE ones-matmul, then exp'd broadcast on ACT
            hlb = headp.tile([P, n], F32, tag="hlb", name="hlb")
            Cb = headp.tile([P, n], BF16, tag="Cb", name="Cb")
            for q in range(n // IB):
                sl = slice(q * IB, (q + 1) * IB)
                ps = ps_prep.tile([P, IB], F32, tag="prep", name="prep")
                nc.tensor.matmul(ps[:], onesr[:], hl_row[0:1, sl], start=True, stop=True)
                nc.vector.tensor_copy(hlb[:, sl], ps[:])
                nc.scalar.activation(Cb[:, sl], ps[:], AF.Exp, scale=ALPHA)

            # ---- hT = W.T @ XTin + b  [P, n] f32 ----
            hT = headp.tile([P, n], F32, tag="hT", name="hT")
            for q in range(n // IB):
                sl = slice(q * IB, (q + 1) * IB)
                ps = ps_prep.tile([P, IB], F32, tag="prep", name="prep")
                nc.tensor.matmul(ps[:], Wc[0][:], XTin[0][:, sl], start=True, stop=False)
                nc.tensor.matmul(ps[:], Wc[1][:], XTin[1][:, sl], start=False, stop=True)
                nc.scalar.activation(hT[:, sl], ps[:], AF.Identity, bias=bcol[:])
            # ---- h chunks [j, d] bf16 via PE transpose ----
            h_bf = []
            for g in range(4):
                tp = ps_prep.tile([P, IB], F32, tag="prep", name="prep")
                for t in range(4):
                    jc = 4 * g + t
                    nc.tensor.transpose(tp[:, t * P:(t + 1) * P],
                                        hT[:, jc * P:(jc + 1) * P], I128[:])
                hg = headp.tile([P, IB], BF16, tag=f"hbf{g}", name=f"hbf{g}")
                nc.vector.tensor_copy(hg[:], tp[:])
                h_bf.append(hg)

            # ---- attention ----
            for half in range(NH):
                i0 = half * HW
                isl = slice(i0, i0 + HW)
                oacc = [ps_main.tile([P, IB], F32, tag=f"oacc{k}", name=f"oacc{k}")
                        for k in range(KH)]
                zacc = [ps_z.tile([2, IB], F32, tag=f"zacc{k}", name=f"zacc{k}")
                        for k in range(KH)]
                for jc in range(NJ):
                    u = up.tile([P, HW], BF16, tag="u", name="u")
                    t1 = t1p.tile([P, HW], BF16, tag="t1", name="t1")
                    nc.scalar.activation(t1[:], hlb[:, isl], AF.Exp,
                                         bias=hrc[:, jc:jc + 1])
                    if jc in T2_ACT:
                        t2 = t1p.tile([P, HW], BF16, tag="t2", name="t2")
                        nc.scalar.activation(t2[:], hlb[:, isl], AF.Exp,
                                             scale=ALPHA, bias=hrc02[:, jc:jc + 1])
                        nc.vector.tensor_tensor(u[:], t1[:], t2[:], AluOpType.max)
                    else:
                        nc.vector.scalar_tensor_tensor(
                            u[:], in0=Cb[:, isl], scalar=Dc[:, jc:jc + 1], in1=t1[:],
                            op0=AluOpType.mult, op1=AluOpType.max)
                    # zero the global diagonal (adjacency excludes self; the
                    # +identity is added post-softmax via v2 = v + hT)
                    dcol = jc * P
                    if i0 <= dcol < i0 + HW:
                        lo = dcol - i0
                        nc.gpsimd.affine_select(
                            out=u[:, lo:lo + P], in_=u[:, lo:lo + P],
                            compare_op=AluOpType.not_equal, fill=0.0,
                            base=0, pattern=[[-1, P]], channel_multiplier=1)
                    for k in range(KH):
                        nc.tensor.matmul(oacc[k][:], h_bf[jc // 4][:, (jc % 4) * P:(jc % 4 + 1) * P],
                                         u[:, k * IB:(k + 1) * IB],
                                         start=(jc == 0), stop=(jc == NJ - 1))
                    for k in range(KH):
                        nc.tensor.matmul(zacc[k][:], ones2[:],
                                         u[:, k * IB:(k + 1) * IB],
                                         start=(jc == 0), stop=(jc == NJ - 1))
                # ---- epilogue for this half ----
                zrow = epp.tile([1, HW], F32, tag="zrow", name="zrow")
                for k in range(KH):
                    nc.vector.tensor_copy(zrow[0:1, k * IB:(k + 1) * IB],
                                          zacc[k][0:1, :])
                zcol = epp.tile([P, HW // P], F32, tag="zcol", name="zcol")
                for q in range(HW // P):
                    nc.sync.dma_start(out=zcol[:, q:q + 1],
                                      in_=zrow[0:1, q * P:(q + 1) * P])
                rcol = epp.tile([P, HW // P], F32, tag="rcol", name="rcol")
                nc.vector.reciprocal(rcol[:], zcol[:])
                rrow = epp.tile([1, HW], F32, tag="rrow", name="rrow")
                for q in range(HW // P):
                    nc.sync.dma_start(out=rrow[0:1, q * P:(q + 1) * P],
                                      in_=rcol[:, q:q + 1])
                for k in range(KH):
                    ksl = slice(i0 + k * IB, i0 + (k + 1) * IB)
                    rb = ps_prep.tile([P, IB], F32, tag="prep", name="prep")
                    nc.tensor.matmul(rb[:], onesr[:],
                                     rrow[0:1, k * IB:(k + 1) * IB].bitcast(F32R),
                                     start=True, stop=True)
                    ob = epp.tile([P, IB], F32, tag="ob", name="ob")
                    nc.scalar.activation(ob[:], oacc[k][:], AF.Copy)
                    v = epp.tile([P, IB], F32, tag="v", name="v")
                    nc.vector.tensor_tensor(v[:], ob[:], rb[:], AluOpType.mult)
                    v2 = epp.tile([P, IB], F32, tag="v2", name="v2")
                    nc.vector.tensor_tensor(v2[:], v[:], hT[:, ksl], AluOpType.add)
                    # y = elu(v2) + 1 = max(v2, 0) + exp(min(v2, 0))
                    m = epp.tile([P, IB], F32, tag="m", name="m")
                    nc.vector.tensor_scalar_min(m[:], v2[:], 0.0)
                    e = epp.tile([P, IB], F32, tag="e", name="e")
                    nc.scalar.activation(e[:], m[:], AF.Exp)
                    nc.vector.scalar_tensor_tensor(
                        OUT[:, ksl], in0=v2[:], scalar=0.0, in1=e[:],
                        op0=AluOpType.max, op1=AluOpType.add)

        def gat_layer(XTin, l, XTout):
            # score rows [4, n]: (hl_h0, hr_h0, hl_h1, hr_h1)
            rows = [headp.tile([2, n], F32R, tag=f"rows{h}", name=f"rows{h}")
                    for h in (0, 1)]
            for q in range(n // IB):
                sl = slice(q * IB, (q + 1) * IB)
                ps = ps_prep.tile([34, IB], F32, tag="prep", name="prep")
                nc.tensor.matmul(ps[:], WAt[l][0][:], XTin[0][:, sl], start=True, stop=False)
                nc.tensor.matmul(ps[:], WAt[l][1][:], XTin[1][:, sl], start=False, stop=True)
                nc.vector.tensor_copy(rows[0][:, sl], ps[0:2, :])
                nc.vector.tensor_copy(rows[1][:, sl], ps[32:34, :])
            for h in (0, 1):
                gat_head(XTin, rows[h][0:1, :], rows[h][1:2, :],
                         Wt[l, h], bt[l, h], cbt[l, h], XTout[h])

        gat_layer(XT, 0, X1T)
        gat_layer(X1T, 1, X2T)

        # ---- transpose X2T back, subtract the elu+1 carry, store ----
        for c in range(NJ):
            ob = smallp.tile([P, F], F32, tag="ost", name="ost")
            for f in range(2):
                tp = ps_prep.tile([P, IB], F32, tag="prep", name="prep")
                nc.tensor.transpose(tp[:, 0:P],
                                    X2T[f][:, c * P:(c + 1) * P].bitcast(F32), I128[:])
                if (c + f) % 2 == 0:
                    nc.vector.tensor_scalar_add(ob[:, f * P:(f + 1) * P], tp[:, 0:P], -1.0)
                else:
                    nc.scalar.activation(ob[:, f * P:(f + 1) * P], tp[:, 0:P],
                                         AF.Copy, bias=-1.0)
            nc.sync.dma_start(out=out_d[c * P:(c + 1) * P, :], in_=ob[:])

    nc.compile()
    return nc


_CACHE = {}
LAST_RESULTS = None


def kernel(**inputs):
    global LAST_RESULTS
    from concourse.bass_utils import run_bass_kernel_spmd

    x = np.ascontiguousarray(np.asarray(inputs["x"], dtype=np.float32))
    B = x.shape[0]
    assert B == N_CORES and x.shape[1] == N and x.shape[2] == F

    if "nc" not in _CACHE:
        _CACHE["nc"] = build_nc()
    nc = _CACHE["nc"]

    base = {}
    for l in (0, 1):
        wa_cols = []
        for h in (0, 1):
            W = np.asarray(inputs[f"W_{l}_{h}"], dtype=np.float64)
            b = np.asarray(inputs[f"b_{l}_{h}"], dtype=np.float64)
            a = np.asarray(inputs[f"a_{l}_{h}"], dtype=np.float64).reshape(-1)
            wa_l = W @ a[:D]
            wa_r = W @ a[D:]
            wa_cols.extend([wa_l, wa_r])
            cb = float(b @ a[:D] + b @ a[D:])
            bb = b.copy()
            if l == 1:
                # layer-2 inputs carry elu+1: x2 = y - 1
                bb = b - W.sum(axis=0)
                cb = cb - float(wa_l.sum() + wa_r.sum())
            base[f"W_{l}_{h}"] = np.ascontiguousarray(W.astype(np.float32))
            base[f"BB_{l}_{h}"] = np.ascontiguousarray(bb.astype(np.float32))
            base[f"CB_{l}_{h}"] = np.array([cb], dtype=np.float32)
        wa_pad = np.zeros((F, 34), dtype=np.float64)
        wa_pad[:, 0] = wa_cols[0]
        wa_pad[:, 1] = wa_cols[1]
        wa_pad[:, 32] = wa_cols[2]
        wa_pad[:, 33] = wa_cols[3]
        base[f"WA_{l}"] = np.ascontiguousarray(wa_pad.astype(np.float32))

    in_maps = [dict(base, x=np.ascontiguousarray(x[i])) for i in range(B)]
    res = run_bass_kernel_spmd(nc, in_maps, list(range(N_CORES)),
                               trace=bool(os.environ.get("BASS_TRACE")))
    LAST_RESULTS = res
    out = np.stack([res.results[i]["out"] for i in range(B)], axis=0)
    return out.astype(np.float32)


# revision 28
# speedup vs baseline: 1.3701x; 1.0445x over previous
"""Dense 2-layer 2-head GAT for Trainium2 (Bass/Tile), data-parallel over batch.

v2 — rank-1 score factorization. Per head the score matrix is
s[i,j] = lrelu(hl_i + hr_j), so

  u = exp(lrelu(s)) = max(exp(s), exp(0.2 s))
    = max(outer(e^{hr}, e^{hl}), outer(e^{0.2 hr}, e^{0.2 hl}))

i.e. an elementwise max of two rank-1 outer products.  hl/hr come from one
tiny [4, n] matmul per layer with host-folded wa = W @ a, so score-tile
generation needs only TWO elementwise passes per tile:

  pass A:  t1 = e^{s}  — either ACT Exp(hl_bcast + hr_j bias)  (per-partition
           bias), or DVE tensor_scalar A_bcast * B_j (4x bf16 mode);
           the per-jc assignment is static to balance ACT vs DVE.
  pass B:  u = DVE stt (C_bcast * D_j) max t1   (2x bf16 mode)

Row-vector broadcasts (hl, A=e^hl, C=e^{0.2 hl}, 1/Z) are materialized by
DMA (idle engine) instead of PE broadcast matmuls.  The diagonal mask is
applied at the source (u diag = 0) by gpsimd affine_select, so no
numerator/denominator correction pass is needed; softmax's +identity is the
v2 = v + hT add.  elu is carried as elu+1 (saves a pass); layer-2 params are
host-adjusted (b2' = b2 - colsum(W2), score-bias consts) and the final store
subtracts 1 during the transpose copy-out.

Z (softmax denominator) is a [128,2]-stationary bf16 ones-matmul on the PE;
u and the h stationary operand are bf16 (scores themselves stay fp32).
"""

import os
from contextlib import ExitStack

import numpy as np

import concourse.bass as bass
import concourse.mybir as mybir
import concourse.tile as tile
from concourse.alu_op_type import AluOpType
from concourse.masks import make_identity

F32 = mybir.dt.float32
F32R = mybir.dt.float32r
BF16 = mybir.dt.bfloat16
AF = mybir.ActivationFunctionType

N = 2048
F = 256
D = 128
P = 128
ALPHA = 0.2
N_CORES = 8

NJ = N // P          # 16 j-chunks
IB = 512             # PSUM bank free width (fp32)
HW = 1024            # i-half width
NH = N // HW         # 2 halves
KH = HW // IB        # 2 k-blocks per half

# jc tiles whose t2 branch is computed on ACT (Exp scale=0.2 w/ bias) and
# combined with a DVE tensor_tensor max (2x bf16); the rest use the DVE
# scalar_tensor_tensor (1x) path.  Tuned to balance ACT vs DVE load.
T2_ACT = (0, 3, 6, 9, 12)


def build_nc(n=N):
    from concourse import bacc
    nc = bacc.Bacc("TRN2", target_bir_lowering=False, debug=False,
                   enable_asserts=False, num_devices=N_CORES)

    x_d = nc.declare_dram_parameter("x", [n, F], F32, isOutput=False)
    W_d, WA_d, BB_d, CB_d = {}, {}, {}, {}
    for l in (0, 1):
        WA_d[l] = nc.declare_dram_parameter(f"WA_{l}", [F, 34], F32, isOutput=False)
        for h in (0, 1):
            W_d[l, h] = nc.declare_dram_parameter(f"W_{l}_{h}", [F, D], F32, isOutput=False)
            BB_d[l, h] = nc.declare_dram_parameter(f"BB_{l}_{h}", [D], F32, isOutput=False)
            CB_d[l, h] = nc.declare_dram_parameter(f"CB_{l}_{h}", [1], F32, isOutput=False)
    out_d = nc.declare_dram_parameter("out", [n, F], F32, isOutput=True)

    with tile.TileContext(nc) as tc, ExitStack() as ctx:
        const = ctx.enter_context(tc.tile_pool(name="const", bufs=1))
        persist = ctx.enter_context(tc.tile_pool(name="persist", bufs=1))
        headp = ctx.enter_context(tc.tile_pool(name="headp", bufs=2))
        up = ctx.enter_context(tc.tile_pool(name="up", bufs=6))
        t1p = ctx.enter_context(tc.tile_pool(name="t1p", bufs=5))
        epp = ctx.enter_context(tc.tile_pool(name="epp", bufs=2))
        smallp = ctx.enter_context(tc.tile_pool(name="smallp", bufs=4))
        ps_prep = ctx.enter_context(tc.tile_pool(name="ps_prep", bufs=2, space="PSUM"))
        ps_main = ctx.enter_context(tc.tile_pool(name="ps_main", bufs=2, space="PSUM"))
        ps_z = ctx.enter_context(tc.tile_pool(name="ps_z", bufs=1, space="PSUM"))

        # ---- constants ----
        I128 = const.tile([P, P], F32, tag="I128", name="I128")
        make_identity(nc, I128[:])
        ones2f = const.tile([P, 2], F32, tag="ones2f", name="ones2f")
        nc.vector.memset(ones2f[:], 1.0)
        ones2 = const.tile([P, 2], BF16, tag="ones2", name="ones2")
        nc.vector.tensor_copy(ones2[:], ones2f[:])
        # ones row [1, P] f32r: stationary of the PE row-broadcast matmul
        onesrf = const.tile([1, P], F32, tag="onesrf", name="onesrf")
        nc.vector.memset(onesrf[:], 1.0)
        onesr = const.tile([1, P], F32R, tag="onesr", name="onesr")
        nc.vector.tensor_copy(onesr[:], onesrf[:])

        # ---- parameters ----
        Wt, WAt, bt, cbt = {}, {}, {}, {}
        for l in (0, 1):
            WAt[l] = []
            for c in range(2):
                waf = smallp.tile([P, 34], F32, tag="waload", name="waload")
                nc.sync.dma_start(out=waf[:], in_=WA_d[l][c * P:(c + 1) * P, :])
                wa = const.tile([P, 34], F32R, tag=f"WA{l}{c}", name=f"WA{l}{c}")
                nc.vector.tensor_copy(wa[:], waf[:])
                WAt[l].append(wa)
            for h in (0, 1):
                Wt[l, h] = []
                for c in range(2):
                    wf = smallp.tile([P, D], F32, tag="wload", name="wload")
                    nc.sync.dma_start(out=wf[:], in_=W_d[l, h][c * P:(c + 1) * P, :])
                    w = const.tile([P, D], F32R, tag=f"W{l}{h}{c}", name=f"W{l}{h}{c}")
                    nc.vector.tensor_copy(w[:], wf[:])
                    Wt[l, h].append(w)
                b = const.tile([P, 1], F32, tag=f"b{l}{h}", name=f"b{l}{h}")
                nc.sync.dma_start(
                    out=b[:], in_=BB_d[l, h][:].rearrange("(p o) -> p o", o=1))
                bt[l, h] = b
                cb = const.tile([P, 1], F32, tag=f"cb{l}{h}", name=f"cb{l}{h}")
                nc.sync.dma_start(
                    out=cb[:],
                    in_=CB_d[l, h][:].rearrange("(o q) -> o q", o=1).to_broadcast([P, 1]))
                cbt[l, h] = cb

        # ---- load x and transpose to XT [2 x (P, n)] f32r ----
        XT = [persist.tile([P, n], F32R, tag=f"XT{f}", name=f"XT{f}") for f in range(2)]
        for c in range(NJ):
            xc = smallp.tile([P, F], F32, tag="xload", name="xload")
            nc.sync.dma_start(out=xc[:], in_=x_d[c * P:(c + 1) * P, :])
            for f in range(2):
                tp = ps_prep.tile([P, IB], F32, tag="prep", name="prep")
                nc.tensor.transpose(tp[:, 0:P], xc[:, f * P:(f + 1) * P], I128[:])
                if (c + f) % 2 == 0:
                    nc.vector.tensor_copy(XT[f][:, c * P:(c + 1) * P], tp[:, 0:P])
                else:
                    nc.scalar.activation(XT[f][:, c * P:(c + 1) * P], tp[:, 0:P], AF.Copy)

        X1T = [persist.tile([P, n], F32R, tag=f"X1T{f}", name=f"X1T{f}") for f in range(2)]
        X2T = XT  # layer-2 output reuses the x tiles (x is consumed by then)

        def head_prep(XTin, hl_row, hr_row, Wc, bcol, cbcol):
            # ---- per-head score vectors ----
            # hr in column layout [P, NJ] via small DMAs, + const bias
            hr_raw = headp.tile([P, NJ], F32, tag="hr_raw", name="hr_raw")
            for jc in range(NJ):
                nc.sync.dma_start(out=hr_raw[:, jc:jc + 1],
                                  in_=hr_row[0:1, jc * P:(jc + 1) * P].bitcast(F32))
            hrc = headp.tile([P, NJ], F32, tag="hrc", name="hrc")
            nc.scalar.activation(hrc[:], hr_raw[:], AF.Identity, bias=cbcol[:])
            Dc = headp.tile([P, NJ], F32, tag="Dc", name="Dc")
            nc.scalar.activation(Dc[:], hrc[:], AF.Exp, scale=ALPHA)
            hrc02 = headp.tile([P, NJ], F32, tag="hrc02", name="hrc02")
            nc.scalar.activation(hrc02[:], hrc[:], AF.Identity, scale=ALPHA)
            # hl broadcast via PE ones-matmul, then exp'd broadcast on ACT
            hlb = headp.tile([P, n], F32, tag="hlb", name="hlb")
            Cb = headp.tile([P, n], BF16, tag="Cb", name="Cb")
            for q in range(n // IB):
                sl = slice(q * IB, (q + 1) * IB)
                ps = ps_prep.tile([P, IB], F32, tag="prep", name="prep")
                nc.tensor.matmul(ps[:], onesr[:], hl_row[0:1, sl], start=True, stop=True)
                nc.vector.tensor_copy(hlb[:, sl], ps[:])
                nc.scalar.activation(Cb[:, sl], ps[:], AF.Exp, scale=ALPHA)

            # ---- hT = W.T @ XTin + b  [P, n] f32 ----
            hT = headp.tile([P, n], F32, tag="hT", name="hT")
            for q in range(n // IB):
                sl = slice(q * IB, (q + 1) * IB)
                ps = ps_prep.tile([P, IB], F32, tag="prep", name="prep")
                nc.tensor.matmul(ps[:], Wc[0][:], XTin[0][:, sl], start=True, stop=False)
                nc.tensor.matmul(ps[:], Wc[1][:], XTin[1][:, sl], start=False, stop=True)
                nc.scalar.activation(hT[:, sl], ps[:], AF.Identity, bias=bcol[:])
            # ---- h chunks [j, d] bf16 via PE transpose ----
            h_bf = []
            for g in range(4):
                tp = ps_prep.tile([P, IB], F32, tag="prep", name="prep")
                for t in range(4):
                    jc = 4 * g + t
                    nc.tensor.transpose(tp[:, t * P:(t + 1) * P],
                                        hT[:, jc * P:(jc + 1) * P], I128[:])
                hg = headp.tile([P, IB], BF16, tag=f"hbf{g}", name=f"hbf{g}")
                nc.vector.tensor_copy(hg[:], tp[:])
                h_bf.append(hg)
            return dict(hrc=hrc, Dc=Dc, hrc02=hrc02, hlb=hlb, Cb=Cb, hT=hT,
                        h_bf=h_bf)

        def head_attn(pp, OUT):
            hrc, Dc, hrc02 = pp["hrc"], pp["Dc"], pp["hrc02"]
            hlb, Cb, hT, h_bf = pp["hlb"], pp["Cb"], pp["hT"], pp["h_bf"]
            for half in range(NH):
                i0 = half * HW
                isl = slice(i0, i0 + HW)
                oacc = [ps_main.tile([P, IB], F32, tag=f"oacc{k}", name=f"oacc{k}")
                        for k in range(KH)]
                zacc = [ps_z.tile([2, IB], F32, tag=f"zacc{k}", name=f"zacc{k}")
                        for k in range(KH)]
                for jc in range(NJ):
                    u = up.tile([P, HW], BF16, tag="u", name="u")
                    t1 = t1p.tile([P, HW], BF16, tag="t1", name="t1")
                    nc.scalar.activation(t1[:], hlb[:, isl], AF.Exp,
                                         bias=hrc[:, jc:jc + 1])
                    if jc in T2_ACT:
                        t2 = t1p.tile([P, HW], BF16, tag="t2", name="t2")
                        nc.scalar.activation(t2[:], hlb[:, isl], AF.Exp,
                                             scale=ALPHA, bias=hrc02[:, jc:jc + 1])
                        nc.vector.tensor_tensor(u[:], t1[:], t2[:], AluOpType.max)
                    else:
                        nc.vector.scalar_tensor_tensor(
                            u[:], in0=Cb[:, isl], scalar=Dc[:, jc:jc + 1], in1=t1[:],
                            op0=AluOpType.mult, op1=AluOpType.max)
                    # zero the global diagonal (adjacency excludes self; the
                    # +identity is added post-softmax via v2 = v + hT)
                    dcol = jc * P
                    if i0 <= dcol < i0 + HW:
                        lo = dcol - i0
                        nc.gpsimd.affine_select(
                            out=u[:, lo:lo + P], in_=u[:, lo:lo + P],
                            compare_op=AluOpType.not_equal, fill=0.0,
                            base=0, pattern=[[-1, P]], channel_multiplier=1)
                    for k in range(KH):
                        nc.tensor.matmul(oacc[k][:], h_bf[jc // 4][:, (jc % 4) * P:(jc % 4 + 1) * P],
                                         u[:, k * IB:(k + 1) * IB],
                                         start=(jc == 0), stop=(jc == NJ - 1))
                    for k in range(KH):
                        nc.tensor.matmul(zacc[k][:], ones2[:],
                                         u[:, k * IB:(k + 1) * IB],
                                         start=(jc == 0), stop=(jc == NJ - 1))
                # ---- epilogue for this half ----
                zrow = epp.tile([1, HW], F32, tag="zrow", name="zrow")
                for k in range(KH):
                    nc.vector.tensor_copy(zrow[0:1, k * IB:(k + 1) * IB],
                                          zacc[k][0:1, :])
                zcol = epp.tile([P, HW // P], F32, tag="zcol", name="zcol")
                for q in range(HW // P):
                    nc.sync.dma_start(out=zcol[:, q:q + 1],
                                      in_=zrow[0:1, q * P:(q + 1) * P])
                rcol = epp.tile([P, HW // P], F32, tag="rcol", name="rcol")
                nc.vector.reciprocal(rcol[:], zcol[:])
                rrow = epp.tile([1, HW], F32, tag="rrow", name="rrow")
                for q in range(HW // P):
                    nc.sync.dma_start(out=rrow[0:1, q * P:(q + 1) * P],
                                      in_=rcol[:, q:q + 1])
                for k in range(KH):
                    ksl = slice(i0 + k * IB, i0 + (k + 1) * IB)
                    rb = ps_prep.tile([P, IB], F32, tag="prep", name="prep")
                    nc.tensor.matmul(rb[:], onesr[:],
                                     rrow[0:1, k * IB:(k + 1) * IB].bitcast(F32R),
                                     start=True, stop=True)
                    ob = epp.tile([P, IB], F32, tag="ob", name="ob")
                    nc.scalar.activation(ob[:], oacc[k][:], AF.Copy)
                    v = epp.tile([P, IB], F32, tag="v", name="v")
                    nc.vector.tensor_tensor(v[:], ob[:], rb[:], AluOpType.mult)
                    v2 = epp.tile([P, IB], F32, tag="v2", name="v2")
                    nc.vector.tensor_tensor(v2[:], v[:], hT[:, ksl], AluOpType.add)
                    # y = elu(v2) + 1 = max(v2, 0) + exp(min(v2, 0))
                    m = epp.tile([P, IB], F32, tag="m", name="m")
                    nc.vector.tensor_scalar_min(m[:], v2[:], 0.0)
                    e = epp.tile([P, IB], F32, tag="e", name="e")
                    nc.scalar.activation(e[:], m[:], AF.Exp)
                    nc.vector.scalar_tensor_tensor(
                        OUT[:, ksl], in0=v2[:], scalar=0.0, in1=e[:],
                        op0=AluOpType.max, op1=AluOpType.add)

        def gat_layer(XTin, l, XTout):
            # score rows [4, n]: (hl_h0, hr_h0, hl_h1, hr_h1)
            rows = [headp.tile([2, n], F32R, tag=f"rows{h}", name=f"rows{h}")
                    for h in (0, 1)]
            for q in range(n // IB):
                sl = slice(q * IB, (q + 1) * IB)
                ps = ps_prep.tile([34, IB], F32, tag="prep", name="prep")
                nc.tensor.matmul(ps[:], WAt[l][0][:], XTin[0][:, sl], start=True, stop=False)
                nc.tensor.matmul(ps[:], WAt[l][1][:], XTin[1][:, sl], start=False, stop=True)
                nc.vector.tensor_copy(rows[0][:, sl], ps[0:2, :])
                nc.vector.tensor_copy(rows[1][:, sl], ps[32:34, :])
            pps = [head_prep(XTin, rows[h][0:1, :], rows[h][1:2, :],
                             Wt[l, h], bt[l, h], cbt[l, h]) for h in (0, 1)]
            for h in (0, 1):
                head_attn(pps[h], XTout[h])

        gat_layer(XT, 0, X1T)
        gat_layer(X1T, 1, X2T)

        # ---- transpose X2T back, subtract the elu+1 carry, store ----
        for c in range(NJ):
            ob = smallp.tile([P, F], F32, tag="ost", name="ost")
            for f in range(2):
                tp = ps_prep.tile([P, IB], F32, tag="prep", name="prep")
                nc.tensor.transpose(tp[:, 0:P],
                                    X2T[f][:, c * P:(c + 1) * P].bitcast(F32), I128[:])
                if (c + f) % 2 == 0:
                    nc.vector.tensor_scalar_add(ob[:, f * P:(f + 1) * P], tp[:, 0:P], -1.0)
                else:
                    nc.scalar.activation(ob[:, f * P:(f + 1) * P], tp[:, 0:P],
                                         AF.Copy, bias=-1.0)
            nc.sync.dma_start(out=out_d[c * P:(c + 1) * P, :], in_=ob[:])

    nc.compile()
    return nc


_CACHE = {}
LAST_RESULTS = None


def kernel(**inputs):
    global LAST_RESULTS
    from concourse.bass_utils import run_bass_kernel_spmd

    x = np.ascontiguousarray(np.asarray(inputs["x"], dtype=np.float32))
    B = x.shape[0]
    assert B == N_CORES and x.shape[1] == N and x.shape[2] == F

    if "nc" not in _CACHE:
        _CACHE["nc"] = build_nc()
    nc = _CACHE["nc"]

    base = {}
    for l in (0, 1):
        wa_cols = []
        for h in (0, 1):
            W = np.asarray(inputs[f"W_{l}_{h}"], dtype=np.float64)
            b = np.asarray(inputs[f"b_{l}_{h}"], dtype=np.float64)
            a = np.asarray(inputs[f"a_{l}_{h}"], dtype=np.float64).reshape(-1)
            wa_l = W @ a[:D]
            wa_r = W @ a[D:]
            wa_cols.extend([wa_l, wa_r])
            cb = float(b @ a[:D] + b @ a[D:])
            bb = b.copy()
            if l == 1:
                # layer-2 inputs carry elu+1: x2 = y - 1
                bb = b - W.sum(axis=0)
                cb = cb - float(wa_l.sum() + wa_r.sum())
            base[f"W_{l}_{h}"] = np.ascontiguousarray(W.astype(np.float32))
            base[f"BB_{l}_{h}"] = np.ascontiguousarray(bb.astype(np.float32))
            base[f"CB_{l}_{h}"] = np.array([cb], dtype=np.float32)
        wa_pad = np.zeros((F, 34), dtype=np.float64)
        wa_pad[:, 0] = wa_cols[0]
        wa_pad[:, 1] = wa_cols[1]
        wa_pad[:, 32] = wa_cols[2]
        wa_pad[:, 33] = wa_cols[3]
        base[f"WA_{l}"] = np.ascontiguousarray(wa_pad.astype(np.float32))

    in_maps = [dict(base, x=np.ascontiguousarray(x[i])) for i in range(B)]
    res = run_bass_kernel_spmd(nc, in_maps, list(range(N_CORES)),
                               trace=bool(os.environ.get("BASS_TRACE")))
    LAST_RESULTS = res
    out = np.stack([res.results[i]["out"] for i in range(B)], axis=0)
    return out.astype(np.float32)
